# revision 19
# baseline (speedup 1.0000x reference)
"""CAGAT MinSum layer (segment-softmax GNN message passing) on 8 TRN2 NeuronCores.

Strategy (v6)
-------------
Algebra: node features are scalars, so the per-edge attention input collapses
to per-head scalar coefficients
    raw[e,k] = a_k*f_src[e] + b_k*f_dst[e] + c_k*m[e] + d_k
    z[e,k]   = exp(lrelu(raw) + p_k*m[e])
and the segment softmax + head-mean + scatter fuses into two segment sums
    Z[n,k] = sum_{e->n} z,   W[n,k] = sum_{e->n} f_src*z
    out[n] = (scaler/H) * sum_k W[n,k] / (Z[n,k] + eps)

Sharding: nodes (and incoming edges) partitioned across 8 cores by dst; each
core owns its output slice, no collectives. Padded-CSR "node-row" layout:
partition p, block b holds one node's edges in a run of W_b columns (widths
mult-of-4, degree-sorted blocks, ~7% pad). Columns are stored QUARTER-SPLIT
(physical order [quarter 0 of all blocks | q1 | q2 | q3]) so the first two
fold levels of every segment sum are full-plane tensor_tensor adds (2 elem/cyc
fp16, one instruction) instead of tensor_reduce (1 elem/cyc). Pads carry
ms=M_BIG so the p*M term drives exp to exactly 0 (host-verified; validity-
plane fallback otherwise).

Device pipeline per head (planes [128, F] fp16, PSUM A = [128, 2048] f32):
  PE  : A = a*fs + b*fd + c*ms (3 diagonal-stationary matmuls per 512-chunk;
        LDWEIGHTS overlaps matmul so per-chunk stationaries are ~free)
  ACT : A = Prelu(A + d_k) in-place (alpha=0.2, one full-plane instr)
  PE  : A += p_k*ms (start=False accumulate onto the ACT output - probed OK)
  ACT : z = Exp(A) -> fp16 (one instr; exp/prelu/ln share one act table set)
  DVE : w = z*fs; fold1/fold2 of the w half; per-width-group reduces batched
        over 4 heads at a time (k-major zg buffer -> mergeable (k t) axis)
  POOL: fold1/fold2 of the z half
ACT alternates Prelu(k+1), Exp(k) so the raw->prelu->pm->exp chain pipelines
across heads. The tail is vectorized over all heads: Ln/Exp gives s8/(Z+eps),
one TT and one head-axis reduce produce the output. Input DMAs issue from the
SP/ACT/Pool queues in parallel; dummy warmup matmuls ramp the PE p-state to
2.4GHz before head 0.

Measured 65.0us on HW (8 cores, +-3us variance), norm rel err 3.5e-4
(vs 76.2us / 2.3e-3 for the previous dual-exp bf16 kernel). Breakdown:
~7.5us fixed runtime/framework preamble + ~3us input DMA, ~38us head loop
(DVE-bound: w-mult + w-half folds + batched reduces; measured HW rates are
TT ~0.67ns/elem, reduce ~1.29ns/elem, Pool ~3.4ns/elem - the documented DVE
2x/4x fp16 modes do not engage for tensor_tensor), ~12us fold/reduce/tail
drain. Further ideas: shift the w-multiply into a second Exp via
w' = exp(g+pm+ln(fs+S)) with tail unmixing W = W'-S*Z (tried: the deeper
per-head chain cost more in pipeline fill than it saved on DVE), or a
64-partition x 2F layout to halve PE's SBUF read traffic.
"""

import sys

sys.path.insert(0, "/opt/trn_rl_repo")

import numpy as np

N_NODES = 50000
N_EDGES = 1600000
HEADS = 8
N_CORES = 8
P = 128
EPS_DEN = 1e-12
M_BIG = 1000.0


# ---------------------------------------------------------------- host prep


def _fold_weights(W_proj, b_proj, W_att, b_att, cycle_penalty, min_sum_scaler):
    H = W_proj.shape[0]
    w = W_proj[:, 0].astype(np.float64)
    Wa = W_att.astype(np.float64)
    a = Wa[:, :H] @ w
    b = Wa[:, H : 2 * H] @ w
    c = Wa[:, 2 * H]
    d = (Wa[:, :H] + Wa[:, H : 2 * H]) @ b_proj.astype(np.float64) + b_att.astype(
        np.float64
    )
    p = cycle_penalty.astype(np.float64)
    s8 = float(min_sum_scaler[0]) / HEADS
    return (
        a.astype(np.float32),
        b.astype(np.float32),
        c.astype(np.float32),
        d.astype(np.float32),
        p.astype(np.float32),
        np.float32(s8),
    )


def _build_layout(dst):
    """Node->(core, partition, block) assignment + unified block widths."""
    n = N_NODES
    deg = np.bincount(dst, minlength=n)
    order = np.argsort(-deg, kind="stable")  # node ids in degree-desc order
    npc = (n + N_CORES - 1) // N_CORES  # nodes per core (6250)
    nb = (npc + P - 1) // P  # blocks per core
    pad_n = npc * N_CORES
    nodes_pad = np.full(pad_n, -1, dtype=np.int64)
    nodes_pad[: len(order)] = order
    node_of = nodes_pad.reshape(npc, N_CORES).T  # [8, npc]

    deg_of = np.where(node_of >= 0, deg[np.clip(node_of, 0, n - 1)], 0)
    pad_npc = nb * P
    deg_pad = np.zeros((N_CORES, pad_npc), dtype=np.int64)
    deg_pad[:, :npc] = deg_of
    blk_max = deg_pad.reshape(N_CORES, nb, P).max(axis=(0, 2))  # [nb]
    W = np.maximum(4, ((blk_max + 3) // 4) * 4).astype(np.int64)  # [nb]
    colbase = np.zeros(nb + 1, dtype=np.int64)
    colbase[1:] = np.cumsum(W)
    F = int(colbase[-1])

    groups = []  # (block_start, count, width, col_offset)
    i = 0
    while i < nb:
        jx = i
        while jx < nb and W[jx] == W[i]:
            jx += 1
        groups.append((i, jx - i, int(W[i]), int(colbase[i])))
        i = jx
    return deg, order, node_of, nb, W, colbase, F, groups


def _edge_cols(dst, layout):
    """Physical (core, partition, quarter-split column) of every edge."""
    deg, order, node_of, nb, W, colbase, F, groups = layout
    n = N_NODES
    rank = np.empty(n, dtype=np.int64)
    rank[order] = np.arange(n)
    core_of_node = rank % N_CORES
    j_of_node = rank // N_CORES
    part_of_node = j_of_node % P
    block_of_node = j_of_node // P

    key = core_of_node[dst] * (node_of.shape[1] + 1) + j_of_node[dst]
    eorder = np.argsort(key, kind="stable")
    dsts = dst[eorder]
    first = np.zeros(len(dsts), dtype=bool)
    first[0] = True
    first[1:] = dsts[1:] != dsts[:-1]
    run_start = np.where(first, np.arange(len(dsts)), 0)
    run_start = np.maximum.accumulate(run_start)
    pos = np.arange(len(dsts)) - run_start

    b = block_of_node[dsts]
    Wq = W[b] // 4  # quarter width of the edge's block
    q = pos // Wq
    r = pos % Wq
    col = q * (F // 4) + colbase[b] // 4 + r
    flat = (core_of_node[dsts] * P + part_of_node[dsts]) * F + col
    return eorder, flat, (core_of_node, j_of_node)


def _build_planes(node_features, cycle_mask, src, dst, layout, need_valid):
    deg, order, node_of, nb, W, colbase, F, groups = layout
    nf = node_features.astype(np.float32)
    eorder, flat, (core_of_node, j_of_node) = _edge_cols(dst, layout)
    srcs = src[eorder]
    msks = cycle_mask[eorder]

    fs = np.zeros(N_CORES * P * F, dtype=np.float32)
    ms = np.full(N_CORES * P * F, M_BIG, dtype=np.float32)
    fs[flat] = nf[srcs]
    ms[flat] = msks
    va = None
    if need_valid:
        va = np.zeros(N_CORES * P * F, dtype=np.float32)
        va[flat] = 1.0
        va = va.reshape(N_CORES, P, F)

    fs = fs.reshape(N_CORES, P, F)
    ms = ms.reshape(N_CORES, P, F)

    # fd plane: per (core, partition, block) = own-node feature, expanded to
    # the block's quarter runs
    fd = np.zeros((N_CORES, P, F), dtype=np.float32)
    nf_blk = np.zeros((N_CORES, P, nb), dtype=np.float32)
    jj = j_of_node
    nf_blk[core_of_node, jj % P, jj // P] = nf
    F4 = F // 4
    for (b0, cnt, Wg, off) in groups:
        seg = np.repeat(nf_blk[:, :, b0 : b0 + cnt], Wg // 4, axis=2)
        for q in range(4):
            o = q * F4 + off // 4
            fd[:, :, o : o + cnt * (Wg // 4)] = seg
    return fs, fd, ms, va


def _check_pad_trick(coef, node_features):
    """lrelu(b*f + c*M + d) + p*M must underflow exp to 0 for every head."""
    a, b, c, d, p, s8 = coef
    f = np.concatenate([node_features.astype(np.float64), [0.0]])
    worst = -np.inf
    for k in range(HEADS):
        t = b[k] * f + c[k] * M_BIG + d[k]
        g = np.maximum(t, 0.2 * t)
        worst = max(worst, float((g + p[k] * M_BIG).max()))
    return worst < -60.0


def _z_dtype(coef, node_features):
    """fp16 for the z/w planes iff the fold sums can't overflow fp16."""
    a, b, c, d, p, s8 = coef
    fmax = float(np.abs(node_features).max())
    U = float(np.max(np.abs(a) * fmax + np.abs(b) * fmax + np.abs(c) + np.abs(d)))
    bound = 4.0 * np.exp(min(U, 50.0)) * max(1.0, fmax)
    return ("float16" if (U < 9.5 and bound < 2.0e4) else "bfloat16"), U


# ------------------------------------------------------------- numpy checker


def _numpy_device_sim(fs, fd, ms, va, coef, layout):
    a, b, c, d, p, s8 = coef
    deg, order, node_of, nb, W, colbase, F, groups = layout
    F2, F4 = F // 2, F // 4
    outs = []
    for ci in range(N_CORES):
        Z = np.zeros((P, HEADS, nb), dtype=np.float32)
        Wn = np.zeros((P, HEADS, nb), dtype=np.float32)
        for k in range(HEADS):
            raw = a[k] * fs[ci] + b[k] * fd[ci] + c[k] * ms[ci] + d[k]
            g = np.where(raw >= 0, raw, 0.2 * raw) + p[k] * ms[ci]
            with np.errstate(over="ignore"):
                z = np.exp(g, dtype=np.float32)
            if va is not None:
                z = z * va[ci]
            w = z * fs[ci]
            for t, plane in enumerate((z, w)):
                pf = plane[:, :F2] + plane[:, F2:]
                pg = pf[:, :F4] + pf[:, F4:]
                for (b0, cnt, Wg, off) in groups:
                    W4 = Wg // 4
                    o4 = off // 4
                    zz = pg[:, o4 : o4 + cnt * W4].reshape(P, cnt, W4)
                    (Z if t == 0 else Wn)[:, k, b0 : b0 + cnt] = zz.sum(axis=2)
        rec = s8 / (Z + np.float32(EPS_DEN))
        outs.append((Wn * rec).sum(axis=1))  # [P, nb]
    return outs


def _assemble(outs, layout):
    deg, order, node_of, nb, W, colbase, F, groups = layout
    npc = node_of.shape[1]
    full = np.zeros(N_NODES, dtype=np.float32)
    jj = np.arange(npc)
    for ci in range(N_CORES):
        vals = outs[ci][jj % P, jj // P]
        nodes = node_of[ci]
        m = nodes >= 0
        full[nodes[m]] = vals[m]
    return full


# ------------------------------------------------------------- bass program


def _build_bass(F, nb, groups, coef, zdt_name, use_valid):
    import concourse.bass as bass
    import concourse.tile as tile
    from concourse import mybir
    import bass_rust

    def _split_excess_waits(nc, max_waits=1):
        """walrus codegen caps sync-wait commands per instruction; move extra
        sem waits onto dedicated same-engine NoOps placed just before."""
        ctr = [0]
        for bb in nc.main_func.blocks:
            new = []
            for ins in bb.instructions:
                si = ins.sync_info
                if si is not None and si.on_wait and len(si.on_wait) > max_waits:
                    waits = list(si.on_wait)
                    si.on_wait = waits[:max_waits]
                    extras = waits[max_waits:]
                    for i in range(0, len(extras), max_waits):
                        ctr[0] += 1
                        nop = mybir.InstNoOp(name=f"waitsplit-{ctr[0]}", ins=[], outs=[])
                        nop.engine = ins.engine
                        nop.sync_info = bass_rust.SyncInfo(
                            on_wait=extras[i : i + max_waits], on_update=[]
                        )
                        nc.register_instruction(nop, overwrite=True)
                        new.append(nop)
                new.append(ins)
            bb.instructions = new

    a, b, c, d, p, s8 = coef
    f32 = mybir.dt.float32
    f16 = mybir.dt.float16
    zdt = getattr(mybir.dt, zdt_name)
    Alu = mybir.AluOpType
    Act = mybir.ActivationFunctionType
    F2, F4 = F // 2, F // 4

    nc = bass.Bass("TRN2")
    fs_d = nc.dram_tensor("fs", [P, F], f16, kind="ExternalInput")
    fd_d = nc.dram_tensor("fd", [P, F], f16, kind="ExternalInput")
    ms_d = nc.dram_tensor("ms", [P, F], f16, kind="ExternalInput")
    dg_d = nc.dram_tensor("dg", [P, (4 * HEADS + 1) * P], f16, kind="ExternalInput")
    if use_valid:
        va_d = nc.dram_tensor("va", [P, F], zdt, kind="ExternalInput")
    out_d = nc.dram_tensor("out", [P, nb], f32, kind="ExternalOutput")

    chunks = []
    off = 0
    while off < F:
        cw = min(512, F - off)
        chunks.append((off, cw))
        off += cw

    with tile.TileContext(nc) as tc:
        with tc.tile_pool(name="pool", bufs=1) as pool, tc.tile_pool(
            name="psum", bufs=2, space="PSUM"
        ) as psum, tc.tile_pool(name="hp", bufs=2) as hp:
            fs = pool.tile([P, F], f16)
            fd = pool.tile([P, F], f16)
            ms = pool.tile([P, F], f16)
            dg = pool.tile([P, (4 * HEADS + 1) * P], f16)
            va = pool.tile([P, F], zdt) if use_valid else None
            # zg is k-major so 4-head reduce batches can merge the (k t) axes
            zg = pool.tile([P, HEADS, 2, F4], zdt)
            sums = pool.tile([P, HEADS, 2, nb], f32)
            bias_t = pool.tile([P, HEADS + 2], f32)
            for k in range(HEADS):
                nc.vector.memset(bias_t[:, k : k + 1], float(d[k]))
            nc.vector.memset(bias_t[:, HEADS : HEADS + 1], float(EPS_DEN))
            nc.vector.memset(
                bias_t[:, HEADS + 1 : HEADS + 2], float(np.log(abs(s8)))
            )

            # loads: first wave (chunk 0 + head-0 stationaries) issued on four
            # idle engines in parallel so head-0 compute starts ~1.5us in;
            # remainders as one large DMA per engine right after
            c0 = slice(0, chunks[0][1])
            early_dmas = []
            early_dmas.append(nc.sync.dma_start(out=fs[:, c0], in_=fs_d[:, c0]))
            early_dmas.append(nc.scalar.dma_start(out=fd[:, c0], in_=fd_d[:, c0]))
            early_dmas.append(nc.gpsimd.dma_start(out=ms[:, c0], in_=ms_d[:, c0]))
            early_dmas.append(
                nc.sync.dma_start(out=dg[:, 0 : 4 * P], in_=dg_d[:, 0 : 4 * P])
            )
            r = slice(chunks[0][1], F)
            early_dmas.append(nc.sync.dma_start(out=fs[:, r], in_=fs_d[:, r]))
            early_dmas.append(nc.scalar.dma_start(out=fd[:, r], in_=fd_d[:, r]))
            early_dmas.append(nc.gpsimd.dma_start(out=ms[:, r], in_=ms_d[:, r]))
            early_dmas.append(
                nc.sync.dma_start(out=dg[:, 4 * P :], in_=dg_d[:, 4 * P :])
            )
            if use_valid:
                early_dmas.append(nc.gpsimd.dma_start(out=va[:], in_=va_d[:]))
            nc._early_input_dmas = early_dmas

            planes = [fs, fd, ms]
            A_of = {}
            zw_of = {}

            # PE p-state warmup: dummy matmuls on a scratch bank while the
            # input DMAs land (results overwritten by head 0's start=True)
            warm = psum.tile([P, 2048], f32, tag="A")
            scratch = pool.tile([P, 512], f16)  # memset, not DMA'd: no DMA dep
            nc.vector.memset(scratch[:], 0.0)
            for _ in range(10):
                nc.tensor.matmul(
                    warm[:, 0:512], lhsT=scratch[:, 0:P], rhs=scratch[:],
                    start=True, stop=True,
                )

            def emit_raw(k):
                A = psum.tile([P, 2048], f32, tag="A")
                A_of[k] = A
                for cf in range(3):
                    lt = dg[:, (k * 4 + cf) * P : (k * 4 + cf + 1) * P]
                    for (off, cw) in chunks:
                        nc.tensor.matmul(
                            A[:, off : off + cw],
                            lhsT=lt,
                            rhs=planes[cf][:, off : off + cw],
                            start=(cf == 0),
                            stop=(cf == 2),
                        )

            def emit_prelu(k):
                A = A_of[k]
                nc.scalar.activation(
                    out=A[:, 0:F], in_=A[:, 0:F], func=Act.Prelu,
                    bias=bias_t[:, k : k + 1], alpha=0.2,
                )

            def emit_pm(k):
                A = A_of[k]
                lt = dg[:, (k * 4 + 3) * P : (k * 4 + 4) * P]
                for (off, cw) in chunks:
                    nc.tensor.matmul(
                        A[:, off : off + cw],
                        lhsT=lt,
                        rhs=ms[:, off : off + cw],
                        start=False,
                        stop=True,
                        skip_group_check=True,
                    )

            def emit_exp(k):
                A = A_of[k]
                zw = hp.tile([P, 2, F], zdt, tag="zw")
                zw_of[k] = zw
                nc.scalar.activation(out=zw[:, 0, :], in_=A[:, 0:F], func=Act.Exp)



            def emit_zpath(k):
                zw = zw_of[k]
                if use_valid:
                    nc.vector.tensor_mul(
                        out=zw[:, 0, :], in0=zw[:, 0, :], in1=va[:]
                    )
                zf = hp.tile([P, 2, F2], zdt, tag="zf")
                # z half on Pool (starts right after Exp), w half on DVE
                nc.gpsimd.tensor_add(
                    out=zf[:, 0, :], in0=zw[:, 0, 0:F2], in1=zw[:, 0, F2:F]
                )
                nc.vector.tensor_mul(out=zw[:, 1, :], in0=zw[:, 0, :], in1=fs[:])
                nc.vector.tensor_add(
                    out=zf[:, 1, :], in0=zw[:, 1, 0:F2], in1=zw[:, 1, F2:F]
                )
                nc.gpsimd.tensor_add(
                    out=zg[:, k, 0, :], in0=zf[:, 0, 0:F4], in1=zf[:, 0, F4:F2]
                )
                nc.vector.tensor_add(
                    out=zg[:, k, 1, :], in0=zf[:, 1, 0:F4], in1=zf[:, 1, F4:F2]
                )

            lnz = pool.tile([P, HEADS, nb], f32)
            rec = pool.tile([P, HEADS, nb], f32)
            prod = pool.tile([P, HEADS, nb], f32)

            def emit_reduce_batch(k0, kn):
                for (b0, cnt, Wg, off) in groups:
                    W4 = Wg // 4
                    o4 = off // 4
                    zin = zg[:, k0 : k0 + kn, :, o4 : o4 + cnt * W4].rearrange(
                        "p k t (c w) -> p (k t) c w", w=W4
                    )
                    nc.vector.tensor_reduce(
                        out=sums[:, k0 : k0 + kn, :, b0 : b0 + cnt], in_=zin,
                        axis=mybir.AxisListType.X, op=Alu.add,
                    )


            # software-pipelined emission: ACT alternates Prelu(k+1), Exp(k)
            emit_raw(0)
            emit_prelu(0)
            emit_raw(1)
            emit_pm(0)
            for k in range(1, HEADS):
                emit_prelu(k)
                emit_exp(k - 1)
                if k + 1 < HEADS:
                    emit_raw(k + 1)
                emit_pm(k)
                emit_zpath(k - 1)
                if k >= 3 and k % 2 == 1:
                    emit_reduce_batch(k - 3, 2)
            emit_exp(HEADS - 1)
            emit_zpath(HEADS - 1)
            emit_reduce_batch(6, 2)

            # tail: rec = s8/(Z+eps) for all heads at once, then head-reduce
            nc.scalar.activation(
                out=lnz[:], in_=sums[:, :, 0, :],
                func=Act.Ln, bias=bias_t[:, HEADS : HEADS + 1],
            )
            nc.scalar.activation(
                out=rec[:], in_=lnz[:], func=Act.Exp, scale=-1.0,
                bias=bias_t[:, HEADS + 1 : HEADS + 2],
            )
            nc.vector.tensor_mul(out=prod[:], in0=sums[:, :, 1, :], in1=rec[:])
            outt = pool.tile([P, nb], f32)
            nc.vector.tensor_reduce(
                out=outt[:], in_=prod[:].rearrange("p k n -> p n k"),
                axis=mybir.AxisListType.X, op=Alu.add,
            )
            if s8 < 0:
                nc.vector.tensor_scalar(
                    out=outt[:], in0=outt[:], scalar1=-1.0, scalar2=None,
                    op0=Alu.mult,
                )
            nc.sync.dma_start(out=out_d[:], in_=outt[:])
    _split_excess_waits(nc)
    # hoist the input DMA issues ahead of the framework preamble so the
    # transfers overlap engine init (they have no upstream dependencies)
    early = getattr(nc, "_early_input_dmas", None)
    if early:
        early_ids = {id(x) for x in early}
        bb = nc.main_func.blocks[0]
        rest = [i for i in bb.instructions if id(i) not in early_ids]
        got = [i for i in bb.instructions if id(i) in early_ids]
        if len(got) == len(early):
            bb.instructions = got + rest
    return nc


# -------------------------------------------------------------------- kernel

_trace_flag = {"trace": False, "last": None}


def kernel(
    node_features,
    cycle_mask,
    W_proj,
    b_proj,
    W_att,
    b_att,
    cycle_penalty,
    min_sum_scaler,
    edge_index,
    _numpy=False,
):
    node_features = np.asarray(node_features)
    cycle_mask = np.asarray(cycle_mask)
    edge_index = np.asarray(edge_index)
    src = edge_index[0].astype(np.int64)
    dst = edge_index[1].astype(np.int64)

    coef = _fold_weights(
        np.asarray(W_proj), np.asarray(b_proj), np.asarray(W_att),
        np.asarray(b_att), np.asarray(cycle_penalty), np.asarray(min_sum_scaler),
    )
    a, b, c, d, p, s8 = coef
    layout = _build_layout(dst)
    deg, order, node_of, nb, W, colbase, F, groups = layout

    if s8 == 0.0:
        return np.zeros(N_NODES, dtype=np.float32)

    use_valid = not _check_pad_trick(coef, node_features)
    fs, fd, ms, va = _build_planes(
        node_features, cycle_mask, src, dst, layout, use_valid
    )

    if _numpy:
        outs = _numpy_device_sim(fs, fd, ms, va, coef, layout)
        return _assemble(outs, layout)

    zdt_name, U = _z_dtype(coef, node_features)

    from concourse.bass_utils import run_bass_kernel_spmd
    import ml_dtypes

    f16 = np.float16
    zdt_np = np.float16 if zdt_name == "float16" else ml_dtypes.bfloat16

    nc = _build_bass(F, nb, groups, coef, zdt_name, use_valid)

    dg = np.zeros((P, (4 * HEADS + 1) * P), dtype=np.float32)
    idx = np.arange(P)
    for k in range(HEADS):
        for cf, cv in enumerate((a[k], b[k], c[k], p[k])):
            dg[idx, (k * 4 + cf) * P + idx] = cv
    dg[idx, 4 * HEADS * P + idx] = 1.0
    dg = dg.astype(f16)
    in_maps = []
    for ci in range(N_CORES):
        m = {
            "fs": fs[ci].astype(f16),
            "fd": fd[ci].astype(f16),
            "ms": ms[ci].astype(f16),
            "dg": dg,
        }
        if use_valid:
            m["va"] = va[ci].astype(zdt_np)
        in_maps.append(m)
    res = run_bass_kernel_spmd(
        nc, in_maps, core_ids=list(range(N_CORES)), trace=_trace_flag["trace"]
    )
    _trace_flag["last"] = res
    outs = [res.results[ci]["out"] for ci in range(N_CORES)]
    return _assemble(outs, layout)
